# revision 21
# baseline (speedup 1.0000x reference)
# Trainium2 Bass kernel for nn_AttentionNeNet (gnn_message_passing).
#
# Math: only the last row r of the context window evolves (each node writes one
# scalar into it), and the output uses only row -1 of (attn @ v). So per node:
#   K_i[j] = sum_f A[j, idx[i,f]] Wk[i,f]   (j < 2047 frozen rows, precomputable)
#   V_i[j] likewise;  q,k_l,v_l = r[idx[i]] @ W[i]
#   out_i = tanh( (sum_j e^{q K_ij - m} V_ij + e^{q k_l - m} v_l) /
#                 (sum_j e^{q K_ij - m}      + e^{q k_l - m}) )
# with m an upper bound of the logits (exact max via precomputed kmax/kmin).
# The node loop is a DAG on in_idxs (node i depends on j<i iff 256+j in idx[i]);
# we level-schedule it (depth ~41 for seed-0 data) and batch nodes per level.
#
# Device layout:
#   K_sb/V_sb: (128p x 16t x 512pos) fp32, T-slot = 128*t + p, node pos on free.
#   Slot (p=0,t=0) is a zero column in A (host pads) and is overwritten per
#   level with the dynamic k_l/v_l, so the last-row term rides along in the
#   den/num reductions for free.
import os
from contextlib import ExitStack

import numpy as np

_IN, _N, _F, _T, _D, _OUT, _C = 256, 512, 32, 2048, 832, 64, 768
_CB = 16  # max nodes per chunk (keeps 16*B <= 256 and 3B <= 64)


def _plan(idx):
    level = np.zeros(_N, np.int64)
    for i in range(_N):
        d = idx[i].astype(np.int64) - _IN
        d = d[(d >= 0) & (d < i)]
        if len(d):
            level[i] = level[d].max() + 1
    order = np.lexsort((np.arange(_N), level))
    pos_of = np.empty(_N, np.int64)
    pos_of[order] = np.arange(_N)
    chunks = []
    off = 0
    for lv in range(int(level.max()) + 1):
        n = int((level == lv).sum())
        s = 0
        while s < n:
            b = min(_CB, n - s)
            # don't let a chunk straddle the pos-256 block boundary
            p = off + s
            if p < 256 and p + b > 256:
                b = 256 - p
            chunks.append((p, b))
            s += b
        off += n
    assert off == _N
    return order, pos_of, chunks


def _host_prep(x, actives, weights, in_idxs):
    x = np.asarray(x, np.float32)
    actives = np.asarray(actives, np.float32)
    W = np.asarray(weights, np.float32)
    idx = np.asarray(in_idxs, np.int64)
    order, pos_of, chunks = _plan(idx)

    # A^T padded: col 0 = zeros (dynamic last-row slot), col 1+j = actives[1+j]
    at = np.zeros((_C, _T), np.float32)
    at[:, 1:] = actives[1:, :_C].T

    # S_kv[c, pos] / S_kv[c, 512+pos]: scatter of Wk/Wv for node order[pos]
    skv = np.zeros((_C, 2 * _N), np.float32)
    rows = idx[order].ravel()                      # (512*32,) c values
    pcol = np.repeat(np.arange(_N), _F)
    np.add.at(skv, (rows, pcol), W[order, :, 1].ravel())
    np.add.at(skv, (rows, _N + pcol), W[order, :, 2].ravel())

    # S2: matvec matrix in u-layout: row uc<256 -> x_uc, row 256+pos -> o[pos].
    # Column layout per chunk (off,B): [q block B | k block B | v block B].
    s2 = np.zeros((_C, 3 * _N), np.float32)
    colq = np.empty(_N, np.int64)
    for off, b in chunks:
        colq[off:off + b] = 3 * off + np.arange(b)
    boff = np.empty(_N, np.int64)  # chunk B for each pos
    for off, b in chunks:
        boff[off:off + b] = b
    for pos in range(_N):
        i = order[pos]
        cq = colq[pos]
        ck = cq + boff[pos]
        cv = cq + 2 * boff[pos]
        for f in range(_F):
            v = idx[i, f]
            if v < _IN:
                row = v
            else:
                j = v - _IN
                if j >= i:
                    continue  # reference reads 0 for self/future nodes
                row = _IN + pos_of[j]
            s2[row, cq] += W[i, f, 0]
            s2[row, ck] += W[i, f, 1]
            s2[row, cv] += W[i, f, 2]

    arrays = {
        "at": at,
        "skv": skv,
        "s2": s2,
        "xrow": x[None, :],                       # (1, 256)
        "xcol": np.ascontiguousarray(x.reshape(2, 128).T),  # (128, 2)
        "ones1": np.ones((1, 128), np.float32),
        "onesc": np.ones((128, 1), np.float32),
        "oneone": np.ones((1, 1), np.float32),
    }
    return arrays, order, pos_of, chunks


def _build(nc, tc, ctx, chunks):
    import concourse.mybir as mybir
    from concourse import bass_isa

    dt = mybir.dt.float32
    AF = mybir.ActivationFunctionType
    OP = mybir.AluOpType
    AX = mybir.AxisListType

    at_d = nc.dram_tensor("at", (_C, _T), dt, kind="ExternalInput").ap()
    skv_d = nc.dram_tensor("skv", (_C, 2 * _N), dt, kind="ExternalInput").ap()
    s2_d = nc.dram_tensor("s2", (_C, 3 * _N), dt, kind="ExternalInput").ap()
    xrow_d = nc.dram_tensor("xrow", (1, _IN), dt, kind="ExternalInput").ap()
    xcol_d = nc.dram_tensor("xcol", (128, 2), dt, kind="ExternalInput").ap()
    ones1_d = nc.dram_tensor("ones1", (1, 128), dt, kind="ExternalInput").ap()
    onesc_d = nc.dram_tensor("onesc", (128, 1), dt, kind="ExternalInput").ap()
    oneone_d = nc.dram_tensor("oneone", (1, 1), dt, kind="ExternalInput").ap()
    out_d = nc.dram_tensor("out", (1, _C), dt, kind="ExternalOutput").ap()

    pool = ctx.enter_context(tc.tile_pool(name="main", bufs=1))
    hv = ctx.enter_context(tc.tile_pool(name="hv", bufs=2))
    rowp = ctx.enter_context(tc.tile_pool(name="rows", bufs=2))
    s2p = ctx.enter_context(tc.tile_pool(name="s2p", bufs=3))

    at_sb = pool.tile([128, 6, _T], dt, tag="at")
    skv_sb = pool.tile([128, 6, 2 * _N], dt, tag="skv")
    k_sb = [pool.tile([128, 16, 256], dt, tag=f"k{b}", name=f"k{b}") for b in range(2)]
    v_sb = [pool.tile([128, 16, 256], dt, tag=f"v{b}", name=f"v{b}") for b in range(2)]
    kmm = [pool.tile([1, 256, 2], dt, tag=f"kmm{b}", name=f"kmm{b}") for b in range(2)]
    u_row = pool.tile([1, _C], dt, tag="urow")
    u_col = pool.tile([128, 6], dt, tag="ucol")
    ones1 = pool.tile([1, 128], dt, tag="ones1")
    onesc = pool.tile([128, 1], dt, tag="onesc")
    oneone = pool.tile([1, 1], dt, tag="oneone")
    kmax = [pool.tile([128, 256], dt, tag=f"kmax{b}", name=f"kmax{b}") for b in range(2)]
    nkmin = [pool.tile([128, 256], dt, tag=f"nkmin{b}", name=f"nkmin{b}") for b in range(2)]

    for ct in range(6):
        nc.sync.dma_start(at_sb[:, ct, :], at_d[128 * ct:128 * (ct + 1), :])
    for ct in range(6):
        nc.sync.dma_start(skv_sb[:, ct, :], skv_d[128 * ct:128 * (ct + 1), :])
    nc.sync.dma_start(ones1, ones1_d)
    nc.sync.dma_start(onesc, onesc_d)
    nc.sync.dma_start(oneone, oneone_d)
    nc.vector.memset(u_row, 0.0)
    nc.sync.dma_start(u_row[0:1, 0:_IN], xrow_d)
    nc.vector.memset(u_col, 0.0)
    nc.sync.dma_start(u_col[:, 0:2], xcol_d)

    # ---- Phase 1: K/V = A @ S (pos-blocked), kmax/kmin over frozen rows ----
    from concourse import library_config
    nc.gpsimd.load_library(library_config.attnmlp)
    ps1 = ctx.enter_context(tc.tile_pool(name="ps1", bufs=2, space="PSUM"))

    def phase1_tile(b, t):
        psk = ps1.tile([128, 256], dt, tag="psk")
        psv = ps1.tile([128, 256], dt, tag="psv")
        for ct in range(6):
            lhs = at_sb[:, ct, 128 * t:128 * (t + 1)]
            nc.tensor.matmul(psk, lhs, skv_sb[:, ct, 256 * b:256 * (b + 1)],
                             start=(ct == 0), stop=(ct == 5))
            nc.tensor.matmul(psv, lhs,
                             skv_sb[:, ct, _N + 256 * b:_N + 256 * (b + 1)],
                             start=(ct == 0), stop=(ct == 5))
        nc.scalar.copy(k_sb[b][:, t, :], psk)
        nc.vector.tensor_copy(v_sb[b][:, t, :], psv)
        if t == 0:
            nc.vector.tensor_copy(kmax[b], psk)
            nc.vector.tensor_scalar_mul(nkmin[b], psk, -1.0)
        else:
            nc.vector.tensor_max(kmax[b], kmax[b], psk)
            nc.vector.scalar_tensor_tensor(nkmin[b], psk, -1.0, nkmin[b],
                                           op0=OP.mult, op1=OP.max)

    def finish_kmm(b):
        nc.gpsimd.partition_all_reduce(kmax[b], kmax[b], channels=128,
                                       reduce_op=bass_isa.ReduceOp.max)
        nc.gpsimd.partition_all_reduce(nkmin[b], nkmin[b], channels=128,
                                       reduce_op=bass_isa.ReduceOp.max)
        nc.vector.tensor_copy(kmm[b][0:1, :, 0], kmax[b][0:1, :])
        nc.scalar.mul(kmm[b][0:1, :, 1], nkmin[b][0:1, :], -1.0)

    for t in range(16):
        phase1_tile(0, t)
    finish_kmm(0)

    # ---- Phase 2: level-scheduled sequential node loop ----
    # Block-1 phase-1 tiles are interleaved between early (pos<256) chunks so
    # the PE crunches them while DVE/ACT walk the serial dependency chain.
    ps_a = ctx.enter_context(tc.tile_pool(name="ps_a", bufs=1, space="PSUM"))
    ps_b = ctx.enter_context(tc.tile_pool(name="ps_b", bufs=1, space="PSUM"))
    b1_t = 0
    for off, B in chunks:
        blk = off // 256
        offl = off % 256
        if blk == 1 and b1_t <= 16:
            while b1_t < 16:
                phase1_tile(1, b1_t)
                b1_t += 1
            finish_kmm(1)
            b1_t = 17
        jmax = (255 + off) // 128 + 1
        s2c = s2p.tile([128, 6, 3 * _CB], dt, tag="s2c")
        nc.sync.dma_start(
            s2c[:, :, 0:3 * B],
            s2_d[:, 3 * off:3 * off + 3 * B].rearrange("(a p) c -> p a c", p=128))

        ps_qkv = ps_a.tile([1, 64], dt, tag="qkv")
        for j in range(jmax):
            nc.tensor.matmul(ps_qkv[0:1, 0:3 * B], u_col[:, j:j + 1],
                             s2c[:, j, 0:3 * B],
                             start=(j == 0), stop=(j == jmax - 1))
        qm = rowp.tile([1, 64], dt, tag="qm")
        nc.scalar.copy(qm[0:1, 0:B], ps_qkv[0:1, 0:B])
        nc.vector.tensor_copy(k_sb[blk][0:1, 0, offl:offl + B],
                              ps_qkv[0:1, B:2 * B])
        nc.vector.tensor_copy(v_sb[blk][0:1, 0, offl:offl + B],
                              ps_qkv[0:1, 2 * B:3 * B])

        # m = max(q*kmax, q*kmin, q*k_l)
        mch = rowp.tile([1, _CB, 2], dt, tag="mch")
        qb2 = ps_qkv[0:1, 0:B].unsqueeze(2).broadcast_to([1, B, 2])
        nc.vector.tensor_mul(mch[0:1, 0:B, :], kmm[blk][0:1, offl:offl + B, :],
                             qb2)
        m12 = rowp.tile([1, _CB], dt, tag="m12")
        nc.vector.reduce_max(m12[0:1, 0:B], mch[0:1, 0:B, :], axis=AX.X)
        t3 = rowp.tile([1, _CB], dt, tag="t3")
        nc.vector.tensor_mul(t3[0:1, 0:B], qm[0:1, 0:B], ps_qkv[0:1, B:2 * B])
        nc.vector.scalar_tensor_tensor(qm[0:1, B:2 * B], t3[0:1, 0:B], 1.0,
                                       m12[0:1, 0:B], op0=OP.mult, op1=OP.max)

        # broadcast [q | m] to all partitions via ones-matmul
        ps_qm = ps_b.tile([128, 32], dt, tag="bc")
        nc.tensor.matmul(ps_qm[:, 0:2 * B], ones1, qm[0:1, 0:2 * B],
                         start=True, stop=True)

        ksl = k_sb[blk][:, :, offl:offl + B]
        q_bc = ps_qm[:, 0:B].unsqueeze(1).broadcast_to([128, 16, B])
        m_bc = ps_qm[:, B:2 * B].unsqueeze(1).broadcast_to([128, 16, B])
        s_scr = hv.tile([128, 16, _CB], dt, tag="s")
        nc.vector.tensor_mul(s_scr[:, :, 0:B], ksl, q_bc)
        s2_scr = hv.tile([128, 16, _CB], dt, tag="s2")
        nc.vector.tensor_sub(s2_scr[:, :, 0:B], s_scr[:, :, 0:B], m_bc)
        e_scr = hv.tile([128, 16, _CB], dt, tag="e")
        nc.scalar.activation(e_scr[:, :, 0:B], s2_scr[:, :, 0:B], AF.Exp)
        ev_scr = hv.tile([128, 16, _CB], dt, tag="s")  # reuse s slot tag
        nc.vector.tensor_mul(ev_scr[:, :, 0:B], e_scr[:, :, 0:B],
                             v_sb[blk][:, :, offl:offl + B])

        ps_dn = ps_b.tile([1, 512], dt, tag="dn")
        nc.tensor.matmul(ps_dn[0:1, 0:16 * B], onesc, e_scr[:, :, 0:B],
                         start=True, stop=True)
        nc.tensor.matmul(ps_dn[0:1, 256:256 + 16 * B], onesc, ev_scr[:, :, 0:B],
                         start=True, stop=True)
        dn = rowp.tile([1, 64], dt, tag="dn")
        pdv = (ps_dn[0:1, :].rearrange("a (u x) -> a u x", u=2)[0:1, :, 0:16 * B]
               .rearrange("a u (t b) -> a u t b", b=B))
        nc.vector.reduce_sum(
            dn[0:1, 0:2 * B], pdv.transpose([0, 1, 3, 2]), axis=AX.X)
        rc = rowp.tile([1, _CB], dt, tag="rc")
        nc.vector.reciprocal(rc[0:1, 0:B], dn[0:1, 0:B])
        rt = rowp.tile([1, _CB], dt, tag="rt")
        nc.vector.tensor_mul(rt[0:1, 0:B], dn[0:1, B:2 * B], rc[0:1, 0:B])
        nc.scalar.activation(u_row[0:1, _IN + off:_IN + off + B],
                             rt[0:1, 0:B], AF.Tanh)

        # refresh u_col blocks touched by this chunk's outputs
        j0 = (_IN + off) // 128
        j1 = (_IN + off + B - 1) // 128
        for j in range(j0, j1 + 1):
            ps_tr = ps_a.tile([128, 1], dt, tag="tr")
            nc.tensor.matmul(ps_tr, u_row[0:1, 128 * j:128 * (j + 1)], oneone,
                             start=True, stop=True)
            nc.vector.tensor_copy(u_col[:, j:j + 1], ps_tr)

        # slip one block-1 phase-1 tile into the PE stream per early chunk
        if blk == 0 and b1_t < 16:
            phase1_tile(1, b1_t)
            b1_t += 1

    nc.sync.dma_start(out_d, u_row)


def make_program(x, actives, weights, in_idxs):
    import concourse.tile as tile
    from concourse import bacc

    arrays, order, pos_of, chunks = _host_prep(x, actives, weights, in_idxs)
    nc = bacc.Bacc("TRN2", target_bir_lowering=False, debug=False,
                   enable_asserts=False, num_devices=8)
    with tile.TileContext(nc) as tc:
        with ExitStack() as ctx:
            _build(nc, tc, ctx, chunks)
    nc.compile()
    return nc, arrays, pos_of


def kernel(x, actives, weights, in_idxs):
    import sys
    if "/opt/trn_rl_repo" not in sys.path:
        sys.path.insert(0, "/opt/trn_rl_repo")
    from concourse.bass_utils import run_bass_kernel_spmd

    nc, arrays, pos_of = make_program(x, actives, weights, in_idxs)
    in_maps = [dict(arrays) for _ in range(8)]
    res = run_bass_kernel_spmd(nc, in_maps, core_ids=list(range(8)))
    u = np.asarray(res.results[0]["out"]).reshape(_C)
    return u[_IN + pos_of[_N - _OUT:_N]].astype(np.float32)


# revision 31
# speedup vs baseline: 1.2012x; 1.2012x over previous
# Trainium2 Bass kernel for nn_AttentionNeNet (gnn_message_passing).
#
# Math: only the last row r of the context window evolves (each node writes one
# scalar into it), and the output uses only row -1 of (attn @ v). So per node:
#   K_i[j] = sum_f A[j, idx[i,f]] Wk[i,f]   (j < 2047 frozen rows, precomputable)
#   V_i[j] likewise;  q,k_l,v_l = r[idx[i]] @ W[i]
#   out_i = tanh( (sum_j e^{q K_ij - m} V_ij + e^{q k_l - m} v_l) /
#                 (sum_j e^{q K_ij - m}      + e^{q k_l - m}) )
# with m an upper bound of the logits (exact max via precomputed kmax/kmin).
# The node loop is a DAG on in_idxs (node i depends on j<i iff 256+j in idx[i]);
# we level-schedule it (depth ~41 for seed-0 data) and batch nodes per level.
#
# Device layout:
#   K_sb/V_sb: (128p x 16t x 512pos) fp32, T-slot = 128*t + p, node pos on free.
#   Slot (p=0,t=0) is a zero column in A (host pads) and is overwritten per
#   level with the dynamic k_l/v_l, so the last-row term rides along in the
#   den/num reductions for free.
import os
from contextlib import ExitStack

import numpy as np

_IN, _N, _F, _T, _D, _OUT, _C = 256, 512, 32, 2048, 832, 64, 768
_CB = 16  # max nodes per chunk (keeps 16*B <= 256 and 3B <= 64)


def _plan(idx):
    level = np.zeros(_N, np.int64)
    for i in range(_N):
        d = idx[i].astype(np.int64) - _IN
        d = d[(d >= 0) & (d < i)]
        if len(d):
            level[i] = level[d].max() + 1
    order = np.lexsort((np.arange(_N), level))
    pos_of = np.empty(_N, np.int64)
    pos_of[order] = np.arange(_N)
    chunks = []
    off = 0
    for lv in range(int(level.max()) + 1):
        n = int((level == lv).sum())
        s = 0
        while s < n:
            b = min(_CB, n - s)
            # don't let a chunk straddle the pos-256 block boundary
            p = off + s
            if p < 256 and p + b > 256:
                b = 256 - p
            chunks.append((p, b))
            s += b
        off += n
    assert off == _N
    return order, pos_of, chunks


def _host_prep(x, actives, weights, in_idxs):
    x = np.asarray(x, np.float32)
    actives = np.asarray(actives, np.float32)
    W = np.asarray(weights, np.float32)
    idx = np.asarray(in_idxs, np.int64)
    order, pos_of, chunks = _plan(idx)

    # A^T padded: col 0 = zeros (dynamic last-row slot), col 1+j = actives[1+j]
    at = np.zeros((_C, _T), np.float32)
    at[:, 1:] = actives[1:, :_C].T

    # S_kv[c, pos] / S_kv[c, 512+pos]: scatter of Wk/Wv for node order[pos]
    skv = np.zeros((_C, 2 * _N), np.float32)
    rows = idx[order].ravel()                      # (512*32,) c values
    pcol = np.repeat(np.arange(_N), _F)
    np.add.at(skv, (rows, pcol), W[order, :, 1].ravel())
    np.add.at(skv, (rows, _N + pcol), W[order, :, 2].ravel())

    # S2: matvec matrix in u-layout: row uc<256 -> x_uc, row 256+pos -> o[pos].
    # Column layout per chunk (off,B): [q block B | k block B | v block B].
    s2 = np.zeros((_C, 3 * _N), np.float32)
    colq = np.empty(_N, np.int64)
    for off, b in chunks:
        colq[off:off + b] = 3 * off + np.arange(b)
    boff = np.empty(_N, np.int64)  # chunk B for each pos
    for off, b in chunks:
        boff[off:off + b] = b
    for pos in range(_N):
        i = order[pos]
        cq = colq[pos]
        ck = cq + boff[pos]
        cv = cq + 2 * boff[pos]
        for f in range(_F):
            v = idx[i, f]
            if v < _IN:
                row = v
            else:
                j = v - _IN
                if j >= i:
                    continue  # reference reads 0 for self/future nodes
                row = _IN + pos_of[j]
            s2[row, cq] += W[i, f, 0]
            s2[row, ck] += W[i, f, 1]
            s2[row, cv] += W[i, f, 2]

    # effective matvec depth per chunk: highest u-tile with any nonzero S2 row
    chunks3 = []
    for off, b in chunks:
        cols = s2[:, 3 * off:3 * off + 3 * b]
        nzr = np.nonzero(np.any(cols != 0.0, axis=1))[0]
        jmax = 2 if len(nzr) == 0 else max(2, int(nzr.max()) // 128 + 1)
        chunks3.append((off, b, jmax))
    chunks = chunks3

    arrays = {
        "at": at,
        "skv": skv,
        "s2": s2,
        "xrow": x[None, :],                       # (1, 256)
        "xcol": np.ascontiguousarray(x.reshape(2, 128).T),  # (128, 2)
        "ones1": np.ones((1, 128), np.float32),
        "onesc": np.ones((128, 1), np.float32),
        "oneone": np.ones((1, 1), np.float32),
    }
    return arrays, order, pos_of, chunks


def _build(nc, tc, ctx, chunks):
    import concourse.mybir as mybir
    from concourse import bass_isa

    dt = mybir.dt.float32
    AF = mybir.ActivationFunctionType
    OP = mybir.AluOpType
    AX = mybir.AxisListType

    at_d = nc.dram_tensor("at", (_C, _T), dt, kind="ExternalInput").ap()
    skv_d = nc.dram_tensor("skv", (_C, 2 * _N), dt, kind="ExternalInput").ap()
    s2_d = nc.dram_tensor("s2", (_C, 3 * _N), dt, kind="ExternalInput").ap()
    xrow_d = nc.dram_tensor("xrow", (1, _IN), dt, kind="ExternalInput").ap()
    xcol_d = nc.dram_tensor("xcol", (128, 2), dt, kind="ExternalInput").ap()
    ones1_d = nc.dram_tensor("ones1", (1, 128), dt, kind="ExternalInput").ap()
    onesc_d = nc.dram_tensor("onesc", (128, 1), dt, kind="ExternalInput").ap()
    oneone_d = nc.dram_tensor("oneone", (1, 1), dt, kind="ExternalInput").ap()
    out_d = nc.dram_tensor("out", (1, _C), dt, kind="ExternalOutput").ap()

    pool = ctx.enter_context(tc.tile_pool(name="main", bufs=1))
    hv = ctx.enter_context(tc.tile_pool(name="hv", bufs=2))
    rowp = ctx.enter_context(tc.tile_pool(name="rows", bufs=2))
    s2p = ctx.enter_context(tc.tile_pool(name="s2p", bufs=3))

    at_sb = pool.tile([128, 6, _T], dt, tag="at")
    skv_sb = pool.tile([128, 6, 2 * _N], dt, tag="skv")
    k_sb = [pool.tile([128, 16, 256], dt, tag=f"k{b}", name=f"k{b}") for b in range(2)]
    v_sb = [pool.tile([128, 16, 256], dt, tag=f"v{b}", name=f"v{b}") for b in range(2)]
    kmm = [pool.tile([128, 256, 2], dt, tag=f"kmm{b}", name=f"kmm{b}") for b in range(2)]
    u_row = pool.tile([1, _C], dt, tag="urow")
    u_col = pool.tile([128, 6], dt, tag="ucol")
    ones1 = pool.tile([1, 128], dt, tag="ones1")
    onesc = pool.tile([128, 1], dt, tag="onesc")
    oneone = pool.tile([1, 1], dt, tag="oneone")
    kmax = [pool.tile([128, 256], dt, tag=f"kmax{b}", name=f"kmax{b}") for b in range(2)]
    nkmin = [pool.tile([128, 256], dt, tag=f"nkmin{b}", name=f"nkmin{b}") for b in range(2)]

    for ct in range(6):
        nc.sync.dma_start(at_sb[:, ct, :], at_d[128 * ct:128 * (ct + 1), :])
    for ct in range(6):
        nc.sync.dma_start(skv_sb[:, ct, :], skv_d[128 * ct:128 * (ct + 1), :])
    nc.sync.dma_start(ones1, ones1_d)
    nc.sync.dma_start(onesc, onesc_d)
    nc.sync.dma_start(oneone, oneone_d)
    nc.vector.memset(u_row, 0.0)
    nc.sync.dma_start(u_row[0:1, 0:_IN], xrow_d)
    nc.vector.memset(u_col, 0.0)
    nc.sync.dma_start(u_col[:, 0:2], xcol_d)

    # ---- Phase 1: K/V = A @ S (pos-blocked), kmax/kmin over frozen rows ----
    from concourse import library_config
    nc.gpsimd.load_library(library_config.attnmlp)
    ps1 = ctx.enter_context(tc.tile_pool(name="ps1", bufs=2, space="PSUM"))

    def phase1_tile(b, t):
        psk = ps1.tile([128, 256], dt, tag="psk")
        psv = ps1.tile([128, 256], dt, tag="psv")
        for ct in range(6):
            lhs = at_sb[:, ct, 128 * t:128 * (t + 1)]
            nc.tensor.matmul(psk, lhs, skv_sb[:, ct, 256 * b:256 * (b + 1)],
                             start=(ct == 0), stop=(ct == 5))
            nc.tensor.matmul(psv, lhs,
                             skv_sb[:, ct, _N + 256 * b:_N + 256 * (b + 1)],
                             start=(ct == 0), stop=(ct == 5))
        nc.scalar.copy(k_sb[b][:, t, :], psk)
        nc.vector.tensor_copy(v_sb[b][:, t, :], psv)
        if t == 0:
            nc.vector.tensor_copy(kmax[b], psk)
            nc.vector.tensor_scalar_mul(nkmin[b], psk, -1.0)
        else:
            nc.vector.tensor_max(kmax[b], kmax[b], psk)
            nc.vector.scalar_tensor_tensor(nkmin[b], psk, -1.0, nkmin[b],
                                           op0=OP.mult, op1=OP.max)

    def finish_kmm(b):
        # partition_all_reduce leaves the result on ALL partitions, so kmm
        # can be built replicated — the whole m-chain then runs on 128 lanes.
        nc.gpsimd.partition_all_reduce(kmax[b], kmax[b], channels=128,
                                       reduce_op=bass_isa.ReduceOp.max)
        nc.gpsimd.partition_all_reduce(nkmin[b], nkmin[b], channels=128,
                                       reduce_op=bass_isa.ReduceOp.max)
        nc.vector.tensor_copy(kmm[b][:, :, 0], kmax[b])
        nc.scalar.mul(kmm[b][:, :, 1], nkmin[b], -1.0)

    # static x-contribution to every node's [q|k|v]: u cols 0,1 never change,
    # so fold them out of the per-chunk matvec into one precomputed table.
    static_qkv = pool.tile([128, 3 * _N], dt, tag="stq")
    with tc.tile_pool(name="s2xp", bufs=1) as s2xp:
        s2x = s2xp.tile([128, 2, 3 * _N], dt, tag="s2x")
        for j in range(2):
            nc.sync.dma_start(s2x[:, j, :], s2_d[128 * j:128 * (j + 1), :])
        with tc.tile_pool(name="psst", bufs=1, space="PSUM") as psst:
            for seg in range(3):
                ps_st = psst.tile([128, 512], dt, tag="ps_st")
                for j in range(2):
                    nc.tensor.matmul(ps_st,
                                     u_col[:, j:j + 1].broadcast_to([128, 128]),
                                     s2x[:, j, 512 * seg:512 * (seg + 1)],
                                     start=(j == 0), stop=(j == 1))
                nc.scalar.copy(static_qkv[:, 512 * seg:512 * (seg + 1)], ps_st)

    for t in range(16):
        phase1_tile(0, t)
    finish_kmm(0)

    # ---- Phase 2: level-scheduled sequential node loop ----
    # Block-1 phase-1 tiles are interleaved between early (pos<256) chunks so
    # the PE crunches them while DVE/ACT walk the serial dependency chain.
    ps_a = ctx.enter_context(tc.tile_pool(name="ps_a", bufs=1, space="PSUM"))
    ps_b = ctx.enter_context(tc.tile_pool(name="ps_b", bufs=1, space="PSUM"))
    b1_t = 0
    for off, B, jmax in chunks:
        blk = off // 256
        offl = off % 256
        if blk == 1 and b1_t <= 16:
            while b1_t < 16:
                phase1_tile(1, b1_t)
                b1_t += 1
            finish_kmm(1)
            b1_t = 17
        s2c = s2p.tile([128, 6, 3 * _CB], dt, tag="s2c")
        if jmax > 2:
            nc.sync.dma_start(
                s2c[:, 2:jmax, 0:3 * B],
                s2_d[256:128 * jmax, 3 * off:3 * off + 3 * B]
                .rearrange("(a p) c -> p a c", p=128))

        # broadcast matvec over the dynamic out-columns only (x part is in
        # static_qkv): stationary u_col column replicated along free, so
        # every partition of the PSUM result holds the same [q|k|v] row.
        qkv = rowp.tile([128, 64], dt, tag="qkv_sb")
        stq = static_qkv[:, 3 * off:3 * off + 3 * B]
        if jmax <= 2:
            nc.scalar.copy(qkv[:, 0:3 * B], stq)
        else:
            ps_qkv = ps_a.tile([128, 64], dt, tag="qkv")
            for j in range(2, jmax):
                nc.tensor.matmul(ps_qkv[:, 0:3 * B],
                                 u_col[:, j:j + 1].broadcast_to([128, 128]),
                                 s2c[:, j, 0:3 * B],
                                 start=(j == 2), stop=(j == jmax - 1))
            nc.vector.scalar_tensor_tensor(qkv[:, 0:3 * B], ps_qkv[:, 0:3 * B],
                                           1.0, stq, op0=OP.mult, op1=OP.add)
        nc.scalar.copy(k_sb[blk][0:1, 0, offl:offl + B], qkv[0:1, B:2 * B])
        nc.scalar.copy(v_sb[blk][0:1, 0, offl:offl + B], qkv[0:1, 2 * B:3 * B])

        # m = max(q*kmax, q*kmin, q*k_l), replicated on all 128 partitions
        mch = rowp.tile([128, _CB, 2], dt, tag="mch")
        qb2 = qkv[:, 0:B].unsqueeze(2).broadcast_to([128, B, 2])
        nc.vector.tensor_mul(mch[:, 0:B, :], kmm[blk][:, offl:offl + B, :], qb2)
        m12 = rowp.tile([128, _CB], dt, tag="m12")
        nc.vector.reduce_max(m12[:, 0:B], mch[:, 0:B, :], axis=AX.X)
        t3 = rowp.tile([128, _CB], dt, tag="t3")
        nc.vector.tensor_mul(t3[:, 0:B], qkv[:, 0:B], qkv[:, B:2 * B])
        mst = rowp.tile([128, _CB], dt, tag="mst")
        nc.vector.scalar_tensor_tensor(mst[:, 0:B], t3[:, 0:B], 1.0,
                                       m12[:, 0:B], op0=OP.mult, op1=OP.max)

        ksl = k_sb[blk][:, :, offl:offl + B]
        q_bc = qkv[:, 0:B].unsqueeze(1).broadcast_to([128, 16, B])
        m_bc = mst[:, 0:B].unsqueeze(1).broadcast_to([128, 16, B])
        s_scr = hv.tile([128, 16, _CB], dt, tag="s")
        nc.vector.tensor_mul(s_scr[:, :, 0:B], ksl, q_bc)
        s2_scr = hv.tile([128, 16, _CB], dt, tag="s2")
        nc.vector.tensor_sub(s2_scr[:, :, 0:B], s_scr[:, :, 0:B], m_bc)
        e_scr = hv.tile([128, 16, _CB], dt, tag="e")
        nc.scalar.activation(e_scr[:, :, 0:B], s2_scr[:, :, 0:B], AF.Exp)
        ev_scr = hv.tile([128, 16, _CB], dt, tag="s")  # reuse s slot tag
        nc.vector.tensor_mul(ev_scr[:, :, 0:B], e_scr[:, :, 0:B],
                             v_sb[blk][:, :, offl:offl + B])

        ps_dn = ps_b.tile([1, 512], dt, tag="dn")
        nc.tensor.matmul(ps_dn[0:1, 0:16 * B], onesc, e_scr[:, :, 0:B],
                         start=True, stop=True)
        nc.tensor.matmul(ps_dn[0:1, 256:256 + 16 * B], onesc, ev_scr[:, :, 0:B],
                         start=True, stop=True)
        dn = rowp.tile([1, 64], dt, tag="dn")
        pdv = (ps_dn[0:1, :].rearrange("a (u x) -> a u x", u=2)[0:1, :, 0:16 * B]
               .rearrange("a u (t b) -> a u t b", b=B))
        nc.vector.reduce_sum(
            dn[0:1, 0:2 * B], pdv.transpose([0, 1, 3, 2]), axis=AX.X)
        rc = rowp.tile([1, _CB], dt, tag="rc")
        nc.vector.reciprocal(rc[0:1, 0:B], dn[0:1, 0:B])
        rt = rowp.tile([1, _CB], dt, tag="rt")
        nc.vector.tensor_mul(rt[0:1, 0:B], dn[0:1, B:2 * B], rc[0:1, 0:B])
        nc.scalar.activation(u_row[0:1, _IN + off:_IN + off + B],
                             rt[0:1, 0:B], AF.Tanh)

        # refresh u_col blocks touched by this chunk's outputs
        j0 = (_IN + off) // 128
        j1 = (_IN + off + B - 1) // 128
        for j in range(j0, j1 + 1):
            ps_tr = ps_a.tile([128, 1], dt, tag="tr")
            nc.tensor.matmul(ps_tr, u_row[0:1, 128 * j:128 * (j + 1)], oneone,
                             start=True, stop=True)
            nc.scalar.copy(u_col[:, j:j + 1], ps_tr)

        # slip one block-1 phase-1 tile into the PE stream per early chunk
        if blk == 0 and b1_t < 16:
            phase1_tile(1, b1_t)
            b1_t += 1

    nc.sync.dma_start(out_d, u_row)


def make_program(x, actives, weights, in_idxs):
    import concourse.tile as tile
    from concourse import bacc

    arrays, order, pos_of, chunks = _host_prep(x, actives, weights, in_idxs)
    nc = bacc.Bacc("TRN2", target_bir_lowering=False, debug=False,
                   enable_asserts=False, num_devices=8)
    with tile.TileContext(nc) as tc:
        with ExitStack() as ctx:
            _build(nc, tc, ctx, chunks)
    nc.compile()
    return nc, arrays, pos_of


def kernel(x, actives, weights, in_idxs):
    import sys
    if "/opt/trn_rl_repo" not in sys.path:
        sys.path.insert(0, "/opt/trn_rl_repo")
    from concourse.bass_utils import run_bass_kernel_spmd

    nc, arrays, pos_of = make_program(x, actives, weights, in_idxs)
    in_maps = [dict(arrays) for _ in range(8)]
    res = run_bass_kernel_spmd(nc, in_maps, core_ids=list(range(8)))
    u = np.asarray(res.results[0]["out"]).reshape(_C)
    return u[_IN + pos_of[_N - _OUT:_N]].astype(np.float32)


# revision 35
# speedup vs baseline: 1.2508x; 1.0413x over previous
# Trainium2 Bass kernel for nn_AttentionNeNet (gnn_message_passing).
#
# Math: only the last row r of the context window evolves (each node writes one
# scalar into it), and the output uses only row -1 of (attn @ v). So per node:
#   K_i[j] = sum_f A[j, idx[i,f]] Wk[i,f]   (j < 2047 frozen rows, precomputable)
#   V_i[j] likewise;  q,k_l,v_l = r[idx[i]] @ W[i]
#   out_i = tanh( (sum_j e^{q K_ij - m} V_ij + e^{q k_l - m} v_l) /
#                 (sum_j e^{q K_ij - m}      + e^{q k_l - m}) )
# with m an upper bound of the logits (exact max via precomputed kmax/kmin).
# The node loop is a DAG on in_idxs (node i depends on j<i iff 256+j in idx[i]);
# we level-schedule it (depth ~41 for seed-0 data) and batch nodes per level.
#
# Device layout:
#   K_sb/V_sb: (128p x 16t x 512pos) fp32, T-slot = 128*t + p, node pos on free.
#   Slot (p=0,t=0) is a zero column in A (host pads) and is overwritten per
#   level with the dynamic k_l/v_l, so the last-row term rides along in the
#   den/num reductions for free.
import os
from contextlib import ExitStack

import numpy as np

_IN, _N, _F, _T, _D, _OUT, _C = 256, 512, 32, 2048, 832, 64, 768
_CB = 16  # max nodes per chunk (keeps 16*B <= 256 and 3B <= 64)


def _plan(idx):
    level = np.zeros(_N, np.int64)
    for i in range(_N):
        d = idx[i].astype(np.int64) - _IN
        d = d[(d >= 0) & (d < i)]
        if len(d):
            level[i] = level[d].max() + 1
    order = np.lexsort((np.arange(_N), level))
    pos_of = np.empty(_N, np.int64)
    pos_of[order] = np.arange(_N)
    chunks = []
    off = 0
    for lv in range(int(level.max()) + 1):
        n = int((level == lv).sum())
        s = 0
        while s < n:
            b = min(_CB, n - s)
            # don't let a chunk straddle the pos-256 block boundary
            p = off + s
            if p < 256 and p + b > 256:
                b = 256 - p
            chunks.append((p, b))
            s += b
        off += n
    assert off == _N
    return order, pos_of, chunks


def _host_prep(x, actives, weights, in_idxs):
    x = np.asarray(x, np.float32)
    actives = np.asarray(actives, np.float32)
    W = np.asarray(weights, np.float32)
    idx = np.asarray(in_idxs, np.int64)
    order, pos_of, chunks = _plan(idx)

    # A^T padded: col 0 = zeros (dynamic last-row slot), col 1+j = actives[1+j]
    at = np.zeros((_C, _T), np.float32)
    at[:, 1:] = actives[1:, :_C].T

    # S_kv[c, pos] / S_kv[c, 512+pos]: scatter of Wk/Wv for node order[pos]
    skv = np.zeros((_C, 2 * _N), np.float32)
    rows = idx[order].ravel()                      # (512*32,) c values
    pcol = np.repeat(np.arange(_N), _F)
    np.add.at(skv, (rows, pcol), W[order, :, 1].ravel())
    np.add.at(skv, (rows, _N + pcol), W[order, :, 2].ravel())

    # S2: matvec matrix in u-layout: row uc<256 -> x_uc, row 256+pos -> o[pos].
    # Column layout per chunk (off,B): [q block B | k block B | v block B].
    s2 = np.zeros((_C, 3 * _N), np.float32)
    colq = np.empty(_N, np.int64)
    for off, b in chunks:
        colq[off:off + b] = 3 * off + np.arange(b)
    boff = np.empty(_N, np.int64)  # chunk B for each pos
    for off, b in chunks:
        boff[off:off + b] = b
    for pos in range(_N):
        i = order[pos]
        cq = colq[pos]
        ck = cq + boff[pos]
        cv = cq + 2 * boff[pos]
        for f in range(_F):
            v = idx[i, f]
            if v < _IN:
                row = v
            else:
                j = v - _IN
                if j >= i:
                    continue  # reference reads 0 for self/future nodes
                row = _IN + pos_of[j]
            s2[row, cq] += W[i, f, 0]
            s2[row, ck] += W[i, f, 1]
            s2[row, cv] += W[i, f, 2]

    # effective matvec depth per chunk: highest u-tile with any nonzero S2 row
    lv_of = np.empty(_N, np.int64)
    lv_of[:] = 0
    lvl = np.zeros(_N, np.int64)
    for i in range(_N):
        d = idx[i] - _IN
        d = d[(d >= 0) & (d < i)]
        if len(d):
            lvl[i] = lvl[d].max() + 1
    chunks3 = []
    for off, b in chunks:
        cols = s2[:, 3 * off:3 * off + 3 * b]
        nzr = np.nonzero(np.any(cols != 0.0, axis=1))[0]
        jmax = 2 if len(nzr) == 0 else max(2, int(nzr.max()) // 128 + 1)
        chunks3.append((off, b, jmax, int(lvl[order[off]])))
    chunks = chunks3

    arrays = {
        "at": at,
        "skv": skv,
        "s2": s2,
        "xrow": x[None, :],                       # (1, 256)
        "xcol": np.ascontiguousarray(x.reshape(2, 128).T),  # (128, 2)
        "ones1": np.ones((1, 128), np.float32),
        "onesc": np.ones((128, 1), np.float32),
        "oneone": np.ones((1, 1), np.float32),
    }
    return arrays, order, pos_of, chunks


def _build(nc, tc, ctx, chunks):
    import concourse.mybir as mybir
    from concourse import bass_isa

    dt = mybir.dt.float32
    AF = mybir.ActivationFunctionType
    OP = mybir.AluOpType
    AX = mybir.AxisListType

    at_d = nc.dram_tensor("at", (_C, _T), dt, kind="ExternalInput").ap()
    skv_d = nc.dram_tensor("skv", (_C, 2 * _N), dt, kind="ExternalInput").ap()
    s2_d = nc.dram_tensor("s2", (_C, 3 * _N), dt, kind="ExternalInput").ap()
    xrow_d = nc.dram_tensor("xrow", (1, _IN), dt, kind="ExternalInput").ap()
    xcol_d = nc.dram_tensor("xcol", (128, 2), dt, kind="ExternalInput").ap()
    ones1_d = nc.dram_tensor("ones1", (1, 128), dt, kind="ExternalInput").ap()
    onesc_d = nc.dram_tensor("onesc", (128, 1), dt, kind="ExternalInput").ap()
    oneone_d = nc.dram_tensor("oneone", (1, 1), dt, kind="ExternalInput").ap()
    out_d = nc.dram_tensor("out", (1, _C), dt, kind="ExternalOutput").ap()

    pool = ctx.enter_context(tc.tile_pool(name="main", bufs=1))
    hv = ctx.enter_context(tc.tile_pool(name="hv", bufs=4))
    rowp = ctx.enter_context(tc.tile_pool(name="rows", bufs=3))
    s2p = ctx.enter_context(tc.tile_pool(name="s2p", bufs=3))

    at_sb = pool.tile([128, 6, _T], dt, tag="at")
    skv_sb = pool.tile([128, 6, 2 * _N], dt, tag="skv")
    k_sb = [pool.tile([128, 16, 256], dt, tag=f"k{b}", name=f"k{b}") for b in range(2)]
    v_sb = [pool.tile([128, 16, 256], dt, tag=f"v{b}", name=f"v{b}") for b in range(2)]
    kmm = [pool.tile([128, 256, 2], dt, tag=f"kmm{b}", name=f"kmm{b}") for b in range(2)]
    u_row = pool.tile([1, _C], dt, tag="urow")
    u_col = pool.tile([128, 6], dt, tag="ucol")
    ones1 = pool.tile([1, 128], dt, tag="ones1")
    onesc = pool.tile([128, 1], dt, tag="onesc")
    oneone = pool.tile([1, 1], dt, tag="oneone")
    kmax = [pool.tile([128, 256], dt, tag=f"kmax{b}", name=f"kmax{b}") for b in range(2)]
    nkmin = [pool.tile([128, 256], dt, tag=f"nkmin{b}", name=f"nkmin{b}") for b in range(2)]

    for ct in range(6):
        nc.sync.dma_start(at_sb[:, ct, :], at_d[128 * ct:128 * (ct + 1), :])
    for ct in range(6):
        nc.sync.dma_start(skv_sb[:, ct, :], skv_d[128 * ct:128 * (ct + 1), :])
    nc.sync.dma_start(ones1, ones1_d)
    nc.sync.dma_start(onesc, onesc_d)
    nc.sync.dma_start(oneone, oneone_d)
    nc.vector.memset(u_row, 0.0)
    nc.sync.dma_start(u_row[0:1, 0:_IN], xrow_d)
    nc.vector.memset(u_col, 0.0)
    nc.sync.dma_start(u_col[:, 0:2], xcol_d)

    # ---- Phase 1: K/V = A @ S (pos-blocked), kmax/kmin over frozen rows ----
    from concourse import library_config
    nc.gpsimd.load_library(library_config.attnmlp)
    ps1 = ctx.enter_context(tc.tile_pool(name="ps1", bufs=2, space="PSUM"))

    def phase1_tile(b, t):
        psk = ps1.tile([128, 256], dt, tag="psk")
        psv = ps1.tile([128, 256], dt, tag="psv")
        for ct in range(6):
            lhs = at_sb[:, ct, 128 * t:128 * (t + 1)]
            nc.tensor.matmul(psk, lhs, skv_sb[:, ct, 256 * b:256 * (b + 1)],
                             start=(ct == 0), stop=(ct == 5))
            nc.tensor.matmul(psv, lhs,
                             skv_sb[:, ct, _N + 256 * b:_N + 256 * (b + 1)],
                             start=(ct == 0), stop=(ct == 5))
        nc.scalar.copy(k_sb[b][:, t, :], psk)
        nc.vector.tensor_copy(v_sb[b][:, t, :], psv)
        if t == 0:
            nc.vector.tensor_copy(kmax[b], psk)
            nc.vector.tensor_scalar_mul(nkmin[b], psk, -1.0)
        else:
            nc.vector.tensor_max(kmax[b], kmax[b], psk)
            nc.vector.scalar_tensor_tensor(nkmin[b], psk, -1.0, nkmin[b],
                                           op0=OP.mult, op1=OP.max)

    def finish_kmm(b):
        # partition_all_reduce leaves the result on ALL partitions, so kmm
        # can be built replicated — the whole m-chain then runs on 128 lanes.
        nc.gpsimd.partition_all_reduce(kmax[b], kmax[b], channels=128,
                                       reduce_op=bass_isa.ReduceOp.max)
        nc.gpsimd.partition_all_reduce(nkmin[b], nkmin[b], channels=128,
                                       reduce_op=bass_isa.ReduceOp.max)
        nc.vector.tensor_copy(kmm[b][:, :, 0], kmax[b])
        nc.scalar.mul(kmm[b][:, :, 1], nkmin[b], -1.0)

    # static x-contribution to every node's [q|k|v]: u cols 0,1 never change,
    # so fold them out of the per-chunk matvec into one precomputed table.
    static_qkv = pool.tile([128, 3 * _N], dt, tag="stq")
    with tc.tile_pool(name="s2xp", bufs=1) as s2xp:
        s2x = s2xp.tile([128, 2, 3 * _N], dt, tag="s2x")
        for j in range(2):
            nc.sync.dma_start(s2x[:, j, :], s2_d[128 * j:128 * (j + 1), :])
        with tc.tile_pool(name="psst", bufs=1, space="PSUM") as psst:
            for seg in range(3):
                ps_st = psst.tile([128, 512], dt, tag="ps_st")
                for j in range(2):
                    nc.tensor.matmul(ps_st,
                                     u_col[:, j:j + 1].broadcast_to([128, 128]),
                                     s2x[:, j, 512 * seg:512 * (seg + 1)],
                                     start=(j == 0), stop=(j == 1))
                nc.scalar.copy(static_qkv[:, 512 * seg:512 * (seg + 1)], ps_st)

    for t in range(16):
        phase1_tile(0, t)
    finish_kmm(0)

    # ---- Phase 2: level-scheduled sequential node loop ----
    # Block-1 phase-1 tiles are interleaved between early (pos<256) chunks so
    # the PE crunches them while DVE/ACT walk the serial dependency chain.
    # Same-level chunk pairs are emitted front-first so the second chunk's
    # matvec isn't stuck behind the first chunk's late PE ops (engine FIFO).
    ps_qv = ctx.enter_context(tc.tile_pool(name="ps_qv", bufs=2, space="PSUM"))
    ps_a = ctx.enter_context(tc.tile_pool(name="ps_a", bufs=1, space="PSUM"))
    ps_b = ctx.enter_context(tc.tile_pool(name="ps_b", bufs=1, space="PSUM"))

    def chunk_front(off, B, jmax):
        # broadcast matvec over the dynamic out-columns only (x part is in
        # static_qkv): stationary u_col column replicated along free, so
        # every partition of the PSUM result holds the same [q|k|v] row.
        s2c = s2p.tile([128, 6, 3 * _CB], dt, tag="s2c", name="s2c")
        if jmax > 2:
            nc.sync.dma_start(
                s2c[:, 2:jmax, 0:3 * B],
                s2_d[256:128 * jmax, 3 * off:3 * off + 3 * B]
                .rearrange("(a p) c -> p a c", p=128))
        qkv = rowp.tile([128, 64], dt, tag="qkv_sb", name="qkv_sb")
        stq = static_qkv[:, 3 * off:3 * off + 3 * B]
        if jmax <= 2:
            nc.scalar.copy(qkv[:, 0:3 * B], stq)
        else:
            ps_qkv = ps_qv.tile([128, 64], dt, tag="qkv", name="ps_qkv")
            for j in range(2, jmax):
                nc.tensor.matmul(ps_qkv[:, 0:3 * B],
                                 u_col[:, j:j + 1].broadcast_to([128, 128]),
                                 s2c[:, j, 0:3 * B],
                                 start=(j == 2), stop=(j == jmax - 1))
            nc.vector.scalar_tensor_tensor(qkv[:, 0:3 * B], ps_qkv[:, 0:3 * B],
                                           1.0, stq, op0=OP.mult, op1=OP.add)
        return qkv

    def chunk_rest(off, B, qkv):
        blk = off // 256
        offl = off % 256
        nc.scalar.copy(k_sb[blk][0:1, 0, offl:offl + B], qkv[0:1, B:2 * B])
        nc.scalar.copy(v_sb[blk][0:1, 0, offl:offl + B], qkv[0:1, 2 * B:3 * B])

        # m = max(q*kmax, q*kmin, q*k_l), replicated on all 128 partitions
        mch = rowp.tile([128, _CB, 2], dt, tag="mch")
        qb2 = qkv[:, 0:B].unsqueeze(2).broadcast_to([128, B, 2])
        nc.vector.tensor_mul(mch[:, 0:B, :], kmm[blk][:, offl:offl + B, :], qb2)
        m12 = rowp.tile([128, _CB], dt, tag="m12")
        nc.vector.reduce_max(m12[:, 0:B], mch[:, 0:B, :], axis=AX.X)
        t3 = rowp.tile([128, _CB], dt, tag="t3")
        nc.vector.tensor_mul(t3[:, 0:B], qkv[:, 0:B], qkv[:, B:2 * B])
        mst = rowp.tile([128, _CB], dt, tag="mst")
        nc.vector.scalar_tensor_tensor(mst[:, 0:B], t3[:, 0:B], 1.0,
                                       m12[:, 0:B], op0=OP.mult, op1=OP.max)

        ksl = k_sb[blk][:, :, offl:offl + B]
        q_bc = qkv[:, 0:B].unsqueeze(1).broadcast_to([128, 16, B])
        m_bc = mst[:, 0:B].unsqueeze(1).broadcast_to([128, 16, B])
        s_scr = hv.tile([128, 16, _CB], dt, tag="s")
        nc.vector.tensor_mul(s_scr[:, :, 0:B], ksl, q_bc)
        s2_scr = hv.tile([128, 16, _CB], dt, tag="s2")
        nc.vector.tensor_sub(s2_scr[:, :, 0:B], s_scr[:, :, 0:B], m_bc)
        e_scr = hv.tile([128, 16, _CB], dt, tag="e")
        nc.scalar.activation(e_scr[:, :, 0:B], s2_scr[:, :, 0:B], AF.Exp)
        ev_scr = hv.tile([128, 16, _CB], dt, tag="s")  # reuse s slot tag
        nc.vector.tensor_mul(ev_scr[:, :, 0:B], e_scr[:, :, 0:B],
                             v_sb[blk][:, :, offl:offl + B])

        ps_dn = ps_b.tile([1, 512], dt, tag="dn")
        nc.tensor.matmul(ps_dn[0:1, 0:16 * B], onesc, e_scr[:, :, 0:B],
                         start=True, stop=True)
        nc.tensor.matmul(ps_dn[0:1, 256:256 + 16 * B], onesc, ev_scr[:, :, 0:B],
                         start=True, stop=True)
        dn = rowp.tile([1, 64], dt, tag="dn")
        pdv = (ps_dn[0:1, :].rearrange("a (u x) -> a u x", u=2)[0:1, :, 0:16 * B]
               .rearrange("a u (t b) -> a u t b", b=B))
        nc.vector.reduce_sum(
            dn[0:1, 0:2 * B], pdv.transpose([0, 1, 3, 2]), axis=AX.X)
        rc = rowp.tile([1, _CB], dt, tag="rc")
        nc.vector.reciprocal(rc[0:1, 0:B], dn[0:1, 0:B])
        rt = rowp.tile([1, _CB], dt, tag="rt")
        nc.vector.tensor_mul(rt[0:1, 0:B], dn[0:1, B:2 * B], rc[0:1, 0:B])
        nc.scalar.activation(u_row[0:1, _IN + off:_IN + off + B],
                             rt[0:1, 0:B], AF.Tanh)

        # refresh u_col blocks touched by this chunk's outputs
        j0 = (_IN + off) // 128
        j1 = (_IN + off + B - 1) // 128
        for j in range(j0, j1 + 1):
            ps_tr = ps_a.tile([128, 1], dt, tag="tr", name="ps_tr")
            nc.tensor.matmul(ps_tr, u_row[0:1, 128 * j:128 * (j + 1)], oneone,
                             start=True, stop=True)
            nc.scalar.copy(u_col[:, j:j + 1], ps_tr)

    b1_t = 0
    i = 0
    while i < len(chunks):
        off, B, jmax, lv = chunks[i]
        blk = off // 256
        if blk == 1 and b1_t <= 16:
            while b1_t < 16:
                phase1_tile(1, b1_t)
                b1_t += 1
            finish_kmm(1)
            b1_t = 17
        pair = None
        if i + 1 < len(chunks):
            off2, B2, jmax2, lv2 = chunks[i + 1]
            if lv2 == lv and off2 // 256 == blk:
                pair = (off2, B2, jmax2)
        qkv1 = chunk_front(off, B, jmax)
        qkv2 = chunk_front(*pair) if pair else None
        chunk_rest(off, B, qkv1)
        if pair:
            chunk_rest(pair[0], pair[1], qkv2)
        # slip one block-1 phase-1 tile into the PE stream per early chunk
        if blk == 0 and b1_t < 16:
            phase1_tile(1, b1_t)
            b1_t += 1
        i += 2 if pair else 1

    nc.sync.dma_start(out_d, u_row)


def make_program(x, actives, weights, in_idxs):
    import concourse.tile as tile
    from concourse import bacc

    arrays, order, pos_of, chunks = _host_prep(x, actives, weights, in_idxs)
    nc = bacc.Bacc("TRN2", target_bir_lowering=False, debug=False,
                   enable_asserts=False, num_devices=8)
    with tile.TileContext(nc) as tc:
        with ExitStack() as ctx:
            _build(nc, tc, ctx, chunks)
    nc.compile()
    return nc, arrays, pos_of


def kernel(x, actives, weights, in_idxs):
    import sys
    if "/opt/trn_rl_repo" not in sys.path:
        sys.path.insert(0, "/opt/trn_rl_repo")
    from concourse.bass_utils import run_bass_kernel_spmd

    nc, arrays, pos_of = make_program(x, actives, weights, in_idxs)
    in_maps = [dict(arrays) for _ in range(8)]
    res = run_bass_kernel_spmd(nc, in_maps, core_ids=list(range(8)))
    u = np.asarray(res.results[0]["out"]).reshape(_C)
    return u[_IN + pos_of[_N - _OUT:_N]].astype(np.float32)


# revision 43
# speedup vs baseline: 1.2745x; 1.0189x over previous
# Trainium2 Bass kernel for nn_AttentionNeNet (gnn_message_passing).
#
# Math: only the last row r of the context window evolves (each node writes one
# scalar into it), and the output uses only row -1 of (attn @ v). So per node:
#   K_i[j] = sum_f A[j, idx[i,f]] Wk[i,f]   (j < 2047 frozen rows, precomputable)
#   V_i[j] likewise;  q,k_l,v_l = r[idx[i]] @ W[i]
#   out_i = tanh( (sum_j e^{q K_ij - m} V_ij + e^{q k_l - m} v_l) /
#                 (sum_j e^{q K_ij - m}      + e^{q k_l - m}) )
# with m an upper bound of the logits (exact max via precomputed kmax/kmin).
# The node loop is a DAG on in_idxs (node i depends on j<i iff 256+j in idx[i]);
# we level-schedule it (depth ~41 for seed-0 data) and batch nodes per level.
#
# Device layout:
#   K_sb/V_sb: (128p x 16t x 512pos) fp32, T-slot = 128*t + p, node pos on free.
#   Slot (p=0,t=0) is a zero column in A (host pads) and is overwritten per
#   level with the dynamic k_l/v_l, so the last-row term rides along in the
#   den/num reductions for free.
import os
from contextlib import ExitStack

import numpy as np

_IN, _N, _F, _T, _D, _OUT, _C = 256, 512, 32, 2048, 832, 64, 768
_CB = 16  # max nodes per chunk (keeps 16*B <= 256 and 3B <= 64)


def _plan(idx):
    level = np.zeros(_N, np.int64)
    for i in range(_N):
        d = idx[i].astype(np.int64) - _IN
        d = d[(d >= 0) & (d < i)]
        if len(d):
            level[i] = level[d].max() + 1
    order = np.lexsort((np.arange(_N), level))
    pos_of = np.empty(_N, np.int64)
    pos_of[order] = np.arange(_N)
    chunks = []
    off = 0
    for lv in range(int(level.max()) + 1):
        n = int((level == lv).sum())
        s = 0
        while s < n:
            b = min(_CB, n - s)
            # don't let a chunk straddle the pos-256 block boundary
            p = off + s
            if p < 256 and p + b > 256:
                b = 256 - p
            chunks.append((p, b))
            s += b
        off += n
    assert off == _N
    return order, pos_of, chunks


def _host_prep(x, actives, weights, in_idxs):
    x = np.asarray(x, np.float32)
    actives = np.asarray(actives, np.float32)
    W = np.asarray(weights, np.float32)
    idx = np.asarray(in_idxs, np.int64)
    order, pos_of, chunks = _plan(idx)

    # A^T padded: col 0 = zeros (dynamic last-row slot), col 1+j = actives[1+j]
    at = np.zeros((_C, _T), np.float32)
    at[:, 1:] = actives[1:, :_C].T

    # S_kv[c, pos] / S_kv[c, 512+pos]: scatter of Wk/Wv for node order[pos]
    skv = np.zeros((_C, 2 * _N), np.float32)
    rows = idx[order].ravel()                      # (512*32,) c values
    pcol = np.repeat(np.arange(_N), _F)
    np.add.at(skv, (rows, pcol), W[order, :, 1].ravel())
    np.add.at(skv, (rows, _N + pcol), W[order, :, 2].ravel())

    # S2: matvec matrix in u-layout: row uc<256 -> x_uc, row 256+pos -> o[pos].
    # Column layout per chunk (off,B): [q block B | k block B | v block B].
    s2 = np.zeros((_C, 3 * _N), np.float32)
    colq = np.empty(_N, np.int64)
    for off, b in chunks:
        colq[off:off + b] = 3 * off + np.arange(b)
    boff = np.empty(_N, np.int64)  # chunk B for each pos
    for off, b in chunks:
        boff[off:off + b] = b
    for pos in range(_N):
        i = order[pos]
        cq = colq[pos]
        ck = cq + boff[pos]
        cv = cq + 2 * boff[pos]
        for f in range(_F):
            v = idx[i, f]
            if v < _IN:
                row = v
            else:
                j = v - _IN
                if j >= i:
                    continue  # reference reads 0 for self/future nodes
                row = _IN + pos_of[j]
            s2[row, cq] += W[i, f, 0]
            s2[row, ck] += W[i, f, 1]
            s2[row, cv] += W[i, f, 2]

    # effective matvec depth per chunk: highest u-tile with any nonzero S2 row
    lv_of = np.empty(_N, np.int64)
    lv_of[:] = 0
    lvl = np.zeros(_N, np.int64)
    for i in range(_N):
        d = idx[i] - _IN
        d = d[(d >= 0) & (d < i)]
        if len(d):
            lvl[i] = lvl[d].max() + 1
    chunks3 = []
    for off, b in chunks:
        cols = s2[:, 3 * off:3 * off + 3 * b]
        nzr = np.nonzero(np.any(cols != 0.0, axis=1))[0]
        jmax = 2 if len(nzr) == 0 else max(2, int(nzr.max()) // 128 + 1)
        chunks3.append((off, b, jmax, int(lvl[order[off]])))
    chunks = chunks3

    arrays = {
        "at": at,
        "skv": skv,
        "s2": s2,
        "xrow": x[None, :],                       # (1, 256)
        "xcol": np.ascontiguousarray(x.reshape(2, 128).T),  # (128, 2)
        "ones1": np.ones((1, 128), np.float32),
        "onesc": np.ones((128, 1), np.float32),
        "oneone": np.ones((1, 1), np.float32),
    }
    return arrays, order, pos_of, chunks


def _build(nc, tc, ctx, chunks):
    import concourse.mybir as mybir
    from concourse import bass_isa

    dt = mybir.dt.float32
    AF = mybir.ActivationFunctionType
    OP = mybir.AluOpType
    AX = mybir.AxisListType

    at_d = nc.dram_tensor("at", (_C, _T), dt, kind="ExternalInput").ap()
    skv_d = nc.dram_tensor("skv", (_C, 2 * _N), dt, kind="ExternalInput").ap()
    s2_d = nc.dram_tensor("s2", (_C, 3 * _N), dt, kind="ExternalInput").ap()
    xrow_d = nc.dram_tensor("xrow", (1, _IN), dt, kind="ExternalInput").ap()
    xcol_d = nc.dram_tensor("xcol", (128, 2), dt, kind="ExternalInput").ap()
    ones1_d = nc.dram_tensor("ones1", (1, 128), dt, kind="ExternalInput").ap()
    onesc_d = nc.dram_tensor("onesc", (128, 1), dt, kind="ExternalInput").ap()
    oneone_d = nc.dram_tensor("oneone", (1, 1), dt, kind="ExternalInput").ap()
    out_d = nc.dram_tensor("out", (1, _C), dt, kind="ExternalOutput").ap()

    pool = ctx.enter_context(tc.tile_pool(name="main", bufs=1))
    hv = ctx.enter_context(tc.tile_pool(name="hv", bufs=4))
    rowp = ctx.enter_context(tc.tile_pool(name="rows", bufs=3))
    s2p = ctx.enter_context(tc.tile_pool(name="s2p", bufs=3))

    at_sb = pool.tile([128, 6, _T], dt, tag="at")
    skv_sb = pool.tile([128, 6, 2 * _N], dt, tag="skv")
    k_sb = [pool.tile([128, 16, 256], dt, tag=f"k{b}", name=f"k{b}") for b in range(2)]
    v_sb = [pool.tile([128, 16, 256], dt, tag=f"v{b}", name=f"v{b}") for b in range(2)]
    kmm = [pool.tile([128, 256, 2], dt, tag=f"kmm{b}", name=f"kmm{b}") for b in range(2)]
    u_row = pool.tile([1, _C], dt, tag="urow")
    u_col = pool.tile([128, 6], dt, tag="ucol")
    ones1 = pool.tile([1, 128], dt, tag="ones1")
    onesc = pool.tile([128, 1], dt, tag="onesc")
    oneone = pool.tile([1, 1], dt, tag="oneone")
    kmax = [pool.tile([128, 256], dt, tag=f"kmax{b}", name=f"kmax{b}") for b in range(2)]
    nkmin = [pool.tile([128, 256], dt, tag=f"nkmin{b}", name=f"nkmin{b}") for b in range(2)]

    for ct in range(6):
        nc.sync.dma_start(at_sb[:, ct, :], at_d[128 * ct:128 * (ct + 1), :])
        nc.sync.dma_start(skv_sb[:, ct, :], skv_d[128 * ct:128 * (ct + 1), :])
    nc.sync.dma_start(ones1, ones1_d)
    nc.sync.dma_start(onesc, onesc_d)
    nc.sync.dma_start(oneone, oneone_d)
    nc.vector.memset(u_row, 0.0)
    nc.sync.dma_start(u_row[0:1, 0:_IN], xrow_d)
    nc.vector.memset(u_col, 0.0)
    nc.sync.dma_start(u_col[:, 0:2], xcol_d)

    # ---- Phase 1: K/V = A @ S (pos-blocked), kmax/kmin over frozen rows ----
    from concourse import library_config
    nc.gpsimd.load_library(library_config.attnmlp)
    ps1 = ctx.enter_context(tc.tile_pool(name="ps1", bufs=2, space="PSUM"))

    def phase1_tile(b, t):
        psk = ps1.tile([128, 256], dt, tag="psk")
        psv = ps1.tile([128, 256], dt, tag="psv")
        for ct in range(6):
            lhs = at_sb[:, ct, 128 * t:128 * (t + 1)]
            nc.tensor.matmul(psk, lhs, skv_sb[:, ct, 256 * b:256 * (b + 1)],
                             start=(ct == 0), stop=(ct == 5))
            nc.tensor.matmul(psv, lhs,
                             skv_sb[:, ct, _N + 256 * b:_N + 256 * (b + 1)],
                             start=(ct == 0), stop=(ct == 5))
        nc.scalar.copy(k_sb[b][:, t, :], psk)
        nc.vector.tensor_copy(v_sb[b][:, t, :], psv)
        if t == 0:
            nc.vector.tensor_copy(kmax[b], psk)
            nc.vector.tensor_scalar_mul(nkmin[b], psk, -1.0)
        else:
            nc.vector.tensor_max(kmax[b], kmax[b], psk)
            nc.vector.scalar_tensor_tensor(nkmin[b], psk, -1.0, nkmin[b],
                                           op0=OP.mult, op1=OP.max)

    def finish_kmm(b):
        # partition_all_reduce leaves the result on ALL partitions, so kmm
        # can be built replicated — the whole m-chain then runs on 128 lanes.
        nc.gpsimd.partition_all_reduce(kmax[b], kmax[b], channels=128,
                                       reduce_op=bass_isa.ReduceOp.max)
        nc.gpsimd.partition_all_reduce(nkmin[b], nkmin[b], channels=128,
                                       reduce_op=bass_isa.ReduceOp.max)
        nc.vector.tensor_copy(kmm[b][:, :, 0], kmax[b])
        nc.scalar.mul(kmm[b][:, :, 1], nkmin[b], -1.0)

    # static x-contribution to every node's [q|k|v]: u cols 0,1 never change,
    # so fold them out of the per-chunk matvec into one precomputed table.
    static_qkv = pool.tile([128, 3 * _N], dt, tag="stq")
    with tc.tile_pool(name="s2xp", bufs=1) as s2xp:
        s2x = s2xp.tile([128, 2, 3 * _N], dt, tag="s2x")
        for j in range(2):
            nc.sync.dma_start(s2x[:, j, :], s2_d[128 * j:128 * (j + 1), :])
        with tc.tile_pool(name="psst", bufs=1, space="PSUM") as psst:
            for seg in range(3):
                ps_st = psst.tile([128, 512], dt, tag="ps_st")
                for j in range(2):
                    nc.tensor.matmul(ps_st,
                                     u_col[:, j:j + 1].broadcast_to([128, 128]),
                                     s2x[:, j, 512 * seg:512 * (seg + 1)],
                                     start=(j == 0), stop=(j == 1))
                nc.scalar.copy(static_qkv[:, 512 * seg:512 * (seg + 1)], ps_st)

    for t in range(16):
        phase1_tile(0, t)
    finish_kmm(0)

    # ---- Phase 2: level-scheduled sequential node loop ----
    # Block-1 phase-1 tiles are interleaved between early (pos<256) chunks so
    # the PE crunches them while DVE/ACT walk the serial dependency chain.
    # Same-level chunk pairs are emitted front-first so the second chunk's
    # matvec isn't stuck behind the first chunk's late PE ops (engine FIFO).
    ps_qv = ctx.enter_context(tc.tile_pool(name="ps_qv", bufs=2, space="PSUM"))
    ps_a = ctx.enter_context(tc.tile_pool(name="ps_a", bufs=1, space="PSUM"))
    ps_b = ctx.enter_context(tc.tile_pool(name="ps_b", bufs=1, space="PSUM"))

    def chunk_front(off, B, jmax):
        # broadcast matvec over the dynamic out-columns only (x part is in
        # static_qkv): stationary u_col column replicated along free, so
        # every partition of the PSUM result holds the same [q|k|v] row.
        s2c = s2p.tile([128, 6, 3 * _CB], dt, tag="s2c", name="s2c")
        if jmax > 2:
            nc.sync.dma_start(
                s2c[:, 2:jmax, 0:3 * B],
                s2_d[256:128 * jmax, 3 * off:3 * off + 3 * B]
                .rearrange("(a p) c -> p a c", p=128))
        qkv = rowp.tile([128, 64], dt, tag="qkv_sb", name="qkv_sb")
        stq = static_qkv[:, 3 * off:3 * off + 3 * B]
        if jmax <= 2:
            nc.scalar.copy(qkv[:, 0:3 * B], stq)
        else:
            ps_qkv = ps_qv.tile([128, 64], dt, tag="qkv", name="ps_qkv")
            for j in range(2, jmax):
                nc.tensor.matmul(ps_qkv[:, 0:3 * B],
                                 u_col[:, j:j + 1].broadcast_to([128, 128]),
                                 s2c[:, j, 0:3 * B],
                                 start=(j == 2), stop=(j == jmax - 1))
            nc.vector.scalar_tensor_tensor(qkv[:, 0:3 * B], ps_qkv[:, 0:3 * B],
                                           1.0, stq, op0=OP.mult, op1=OP.add)
        return qkv

    def chunk_rest(off, B, qkv):
        blk = off // 256
        offl = off % 256
        nc.scalar.copy(k_sb[blk][0:1, 0, offl:offl + B], qkv[0:1, B:2 * B])
        nc.scalar.copy(v_sb[blk][0:1, 0, offl:offl + B], qkv[0:1, 2 * B:3 * B])

        # m = max(q*kmax, q*kmin, q*k_l), replicated on all 128 partitions
        mch = rowp.tile([128, _CB, 2], dt, tag="mch")
        qb2 = qkv[:, 0:B].unsqueeze(2).broadcast_to([128, B, 2])
        nc.vector.tensor_mul(mch[:, 0:B, :], kmm[blk][:, offl:offl + B, :], qb2)
        m12 = rowp.tile([128, _CB], dt, tag="m12")
        nc.vector.reduce_max(m12[:, 0:B], mch[:, 0:B, :], axis=AX.X)
        t3 = rowp.tile([128, _CB], dt, tag="t3")
        nc.vector.tensor_mul(t3[:, 0:B], qkv[:, 0:B], qkv[:, B:2 * B])
        mst = rowp.tile([128, _CB], dt, tag="mst")
        nc.vector.scalar_tensor_tensor(mst[:, 0:B], t3[:, 0:B], 1.0,
                                       m12[:, 0:B], op0=OP.mult, op1=OP.max)

        ksl = k_sb[blk][:, :, offl:offl + B]
        q_bc = qkv[:, 0:B].unsqueeze(1).broadcast_to([128, 16, B])
        m_bc = mst[:, 0:B].unsqueeze(1).broadcast_to([128, 16, B])
        s_scr = hv.tile([128, 16, _CB], dt, tag="s")
        nc.vector.tensor_mul(s_scr[:, :, 0:B], ksl, q_bc)
        s2_scr = hv.tile([128, 16, _CB], dt, tag="s2")
        nc.vector.tensor_sub(s2_scr[:, :, 0:B], s_scr[:, :, 0:B], m_bc)
        e_scr = hv.tile([128, 16, _CB], dt, tag="e")
        nc.scalar.activation(e_scr[:, :, 0:B], s2_scr[:, :, 0:B], AF.Exp)
        ev_scr = hv.tile([128, 16, _CB], dt, tag="s")  # reuse s slot tag
        nc.vector.tensor_mul(ev_scr[:, :, 0:B], e_scr[:, :, 0:B],
                             v_sb[blk][:, :, offl:offl + B])

        ps_dn = ps_b.tile([1, 512], dt, tag="dn")
        nc.tensor.matmul(ps_dn[0:1, 0:16 * B], onesc, e_scr[:, :, 0:B],
                         start=True, stop=True)
        nc.tensor.matmul(ps_dn[0:1, 256:256 + 16 * B], onesc, ev_scr[:, :, 0:B],
                         start=True, stop=True)
        dn = rowp.tile([1, 64], dt, tag="dn")
        pdv = (ps_dn[0:1, :].rearrange("a (u x) -> a u x", u=2)[0:1, :, 0:16 * B]
               .rearrange("a u (t b) -> a u t b", b=B))
        nc.vector.reduce_sum(
            dn[0:1, 0:2 * B], pdv.transpose([0, 1, 3, 2]), axis=AX.X)
        rc = rowp.tile([1, _CB], dt, tag="rc")
        nc.vector.reciprocal(rc[0:1, 0:B], dn[0:1, 0:B])
        rt = rowp.tile([1, _CB], dt, tag="rt")
        nc.vector.tensor_mul(rt[0:1, 0:B], dn[0:1, B:2 * B], rc[0:1, 0:B])
        nc.scalar.activation(u_row[0:1, _IN + off:_IN + off + B],
                             rt[0:1, 0:B], AF.Tanh)

        # refresh u_col blocks touched by this chunk's outputs
        j0 = (_IN + off) // 128
        j1 = (_IN + off + B - 1) // 128
        for j in range(j0, j1 + 1):
            ps_tr = ps_a.tile([128, 1], dt, tag="tr", name="ps_tr")
            nc.tensor.matmul(ps_tr, u_row[0:1, 128 * j:128 * (j + 1)], oneone,
                             start=True, stop=True)
            nc.scalar.copy(u_col[:, j:j + 1], ps_tr)

    b1_t = 0
    i = 0
    while i < len(chunks):
        off, B, jmax, lv = chunks[i]
        blk = off // 256
        if blk == 1 and b1_t <= 16:
            while b1_t < 16:
                phase1_tile(1, b1_t)
                b1_t += 1
            finish_kmm(1)
            b1_t = 17
        pair = None
        if i + 1 < len(chunks):
            off2, B2, jmax2, lv2 = chunks[i + 1]
            if lv2 == lv and off2 // 256 == blk:
                pair = (off2, B2, jmax2)
        qkv1 = chunk_front(off, B, jmax)
        qkv2 = chunk_front(*pair) if pair else None
        chunk_rest(off, B, qkv1)
        if pair:
            chunk_rest(pair[0], pair[1], qkv2)
        # slip one block-1 phase-1 tile into the PE stream per early chunk
        if blk == 0 and b1_t < 16:
            phase1_tile(1, b1_t)
            b1_t += 1
        i += 2 if pair else 1

    nc.sync.dma_start(out_d, u_row)


def make_program(x, actives, weights, in_idxs):
    import concourse.tile as tile
    from concourse import bacc

    arrays, order, pos_of, chunks = _host_prep(x, actives, weights, in_idxs)
    nc = bacc.Bacc("TRN2", target_bir_lowering=False, debug=False,
                   enable_asserts=False, num_devices=8)
    with tile.TileContext(nc) as tc:
        with ExitStack() as ctx:
            _build(nc, tc, ctx, chunks)
    nc.compile()
    return nc, arrays, pos_of


def kernel(x, actives, weights, in_idxs):
    import sys
    if "/opt/trn_rl_repo" not in sys.path:
        sys.path.insert(0, "/opt/trn_rl_repo")
    from concourse.bass_utils import run_bass_kernel_spmd

    nc, arrays, pos_of = make_program(x, actives, weights, in_idxs)
    in_maps = [dict(arrays) for _ in range(8)]
    res = run_bass_kernel_spmd(nc, in_maps, core_ids=list(range(8)))
    u = np.asarray(res.results[0]["out"]).reshape(_C)
    return u[_IN + pos_of[_N - _OUT:_N]].astype(np.float32)


# revision 49
# speedup vs baseline: 1.3192x; 1.0351x over previous
# Trainium2 Bass kernel for nn_AttentionNeNet (gnn_message_passing).
#
# Math: only the last row r of the context window evolves (each node writes one
# scalar into it), and the output uses only row -1 of (attn @ v). So per node:
#   K_i[j] = sum_f A[j, idx[i,f]] Wk[i,f]   (j < 2047 frozen rows, precomputable)
#   V_i[j] likewise;  q,k_l,v_l = r[idx[i]] @ W[i]
#   out_i = tanh( (sum_j e^{q K_ij - m} V_ij + e^{q k_l - m} v_l) /
#                 (sum_j e^{q K_ij - m}      + e^{q k_l - m}) )
# with m an upper bound of the logits (exact max via precomputed kmax/kmin).
# The node loop is a DAG on in_idxs (node i depends on j<i iff 256+j in idx[i]);
# we level-schedule it (depth ~41 for seed-0 data) and batch nodes per level.
#
# Device layout:
#   K_sb/V_sb: (128p x 16t x 512pos) fp32, T-slot = 128*t + p, node pos on free.
#   Slot (p=0,t=0) is a zero column in A (host pads) and is overwritten per
#   level with the dynamic k_l/v_l, so the last-row term rides along in the
#   den/num reductions for free.
import os
from contextlib import ExitStack

import numpy as np

_IN, _N, _F, _T, _D, _OUT, _C = 256, 512, 32, 2048, 832, 64, 768
_CB = 16  # max nodes per chunk (keeps 16*B <= 256 and 3B <= 64)


def _plan(idx):
    level = np.zeros(_N, np.int64)
    for i in range(_N):
        d = idx[i].astype(np.int64) - _IN
        d = d[(d >= 0) & (d < i)]
        if len(d):
            level[i] = level[d].max() + 1
    order = np.lexsort((np.arange(_N), level))
    pos_of = np.empty(_N, np.int64)
    pos_of[order] = np.arange(_N)
    chunks = []
    off = 0
    for lv in range(int(level.max()) + 1):
        n = int((level == lv).sum())
        s = 0
        while s < n:
            b = min(_CB, n - s)
            # don't let a chunk straddle the pos-256 block boundary
            p = off + s
            if p < 256 and p + b > 256:
                b = 256 - p
            chunks.append((p, b))
            s += b
        off += n
    assert off == _N
    return order, pos_of, chunks


def _host_prep(x, actives, weights, in_idxs):
    x = np.asarray(x, np.float32)
    actives = np.asarray(actives, np.float32)
    W = np.asarray(weights, np.float32)
    idx = np.asarray(in_idxs, np.int64)
    order, pos_of, chunks = _plan(idx)

    # A^T padded: col 0 = zeros (dynamic last-row slot), col 1+j = actives[1+j]
    at = np.zeros((_C, _T), np.float32)
    at[:, 1:] = actives[1:, :_C].T

    # S_kv[c, pos] / S_kv[c, 512+pos]: scatter of Wk/Wv for node order[pos]
    skv = np.zeros((_C, 2 * _N), np.float32)
    rows = idx[order].ravel()                      # (512*32,) c values
    pcol = np.repeat(np.arange(_N), _F)
    np.add.at(skv, (rows, pcol), W[order, :, 1].ravel())
    np.add.at(skv, (rows, _N + pcol), W[order, :, 2].ravel())

    # S2: matvec matrix in u-layout: row uc<256 -> x_uc, row 256+pos -> o[pos].
    # Column layout per chunk (off,B): [q block B | k block B | v block B].
    s2 = np.zeros((_C, 3 * _N), np.float32)
    colq = np.empty(_N, np.int64)
    for off, b in chunks:
        colq[off:off + b] = 3 * off + np.arange(b)
    boff = np.empty(_N, np.int64)  # chunk B for each pos
    for off, b in chunks:
        boff[off:off + b] = b
    for pos in range(_N):
        i = order[pos]
        cq = colq[pos]
        ck = cq + boff[pos]
        cv = cq + 2 * boff[pos]
        for f in range(_F):
            v = idx[i, f]
            if v < _IN:
                row = v
            else:
                j = v - _IN
                if j >= i:
                    continue  # reference reads 0 for self/future nodes
                row = _IN + pos_of[j]
            s2[row, cq] += W[i, f, 0]
            s2[row, ck] += W[i, f, 1]
            s2[row, cv] += W[i, f, 2]

    # effective matvec depth per chunk: highest u-tile with any nonzero S2 row
    lv_of = np.empty(_N, np.int64)
    lv_of[:] = 0
    lvl = np.zeros(_N, np.int64)
    for i in range(_N):
        d = idx[i] - _IN
        d = d[(d >= 0) & (d < i)]
        if len(d):
            lvl[i] = lvl[d].max() + 1
    chunks3 = []
    for off, b in chunks:
        cols = s2[:, 3 * off:3 * off + 3 * b]
        nzr = np.nonzero(np.any(cols != 0.0, axis=1))[0]
        jmax = 2 if len(nzr) == 0 else max(2, int(nzr.max()) // 128 + 1)
        chunks3.append((off, b, jmax, int(lvl[order[off]])))
    chunks = chunks3

    arrays = {
        "at": at,
        "skv": skv,
        "s2": s2,
        "xrow": x[None, :],                       # (1, 256)
        "xcol": np.ascontiguousarray(x.reshape(2, 128).T),  # (128, 2)
        "ones1": np.ones((1, 128), np.float32),
        "onesc": np.ones((128, 1), np.float32),
        "oneone": np.ones((1, 1), np.float32),
    }
    return arrays, order, pos_of, chunks


def _build(nc, tc, ctx, chunks):
    import concourse.mybir as mybir
    from concourse import bass_isa

    dt = mybir.dt.float32
    AF = mybir.ActivationFunctionType
    OP = mybir.AluOpType
    AX = mybir.AxisListType

    at_d = nc.dram_tensor("at", (_C, _T), dt, kind="ExternalInput").ap()
    skv_d = nc.dram_tensor("skv", (_C, 2 * _N), dt, kind="ExternalInput").ap()
    s2_d = nc.dram_tensor("s2", (_C, 3 * _N), dt, kind="ExternalInput").ap()
    xrow_d = nc.dram_tensor("xrow", (1, _IN), dt, kind="ExternalInput").ap()
    xcol_d = nc.dram_tensor("xcol", (128, 2), dt, kind="ExternalInput").ap()
    ones1_d = nc.dram_tensor("ones1", (1, 128), dt, kind="ExternalInput").ap()
    onesc_d = nc.dram_tensor("onesc", (128, 1), dt, kind="ExternalInput").ap()
    oneone_d = nc.dram_tensor("oneone", (1, 1), dt, kind="ExternalInput").ap()
    out_d = nc.dram_tensor("out", (1, _C), dt, kind="ExternalOutput").ap()

    pool = ctx.enter_context(tc.tile_pool(name="main", bufs=1))
    hv = ctx.enter_context(tc.tile_pool(name="hv", bufs=4))
    rowp = ctx.enter_context(tc.tile_pool(name="rows", bufs=3))
    s2p = ctx.enter_context(tc.tile_pool(name="s2p", bufs=3))

    at_sb = pool.tile([128, 6, _T], dt, tag="at")
    skv_sb = pool.tile([128, 6, 2 * _N], dt, tag="skv")
    k_sb = [pool.tile([128, 16, 256], dt, tag=f"k{b}", name=f"k{b}") for b in range(2)]
    v_sb = [pool.tile([128, 16, 256], dt, tag=f"v{b}", name=f"v{b}") for b in range(2)]
    kmm = [pool.tile([128, 256, 2], dt, tag=f"kmm{b}", name=f"kmm{b}") for b in range(2)]
    u_row = pool.tile([1, _C], dt, tag="urow")
    u_col = pool.tile([128, 6], dt, tag="ucol")
    ones1 = pool.tile([1, 128], dt, tag="ones1")
    onesc = pool.tile([128, 1], dt, tag="onesc")
    oneone = pool.tile([1, 1], dt, tag="oneone")
    kmax = [pool.tile([128, 256], dt, tag=f"kmax{b}", name=f"kmax{b}") for b in range(2)]
    nkmin = [pool.tile([128, 256], dt, tag=f"nkmin{b}", name=f"nkmin{b}") for b in range(2)]

    # first wave: only the bytes block-0's early matmuls read (t<8 columns of
    # A^T, block-0 K/V columns of S_kv); the rest lands during compute
    for ct in range(6):
        r = slice(128 * ct, 128 * (ct + 1))
        nc.sync.dma_start(at_sb[:, ct, 0:1024], at_d[r, 0:1024])
        nc.sync.dma_start(skv_sb[:, ct, 0:256], skv_d[r, 0:256])
        nc.sync.dma_start(skv_sb[:, ct, 512:768], skv_d[r, 512:768])
    for ct in range(6):
        r = slice(128 * ct, 128 * (ct + 1))
        nc.sync.dma_start(at_sb[:, ct, 1024:2048], at_d[r, 1024:2048])
        nc.sync.dma_start(skv_sb[:, ct, 256:512], skv_d[r, 256:512])
        nc.sync.dma_start(skv_sb[:, ct, 768:1024], skv_d[r, 768:1024])
    nc.sync.dma_start(ones1, ones1_d)
    nc.sync.dma_start(onesc, onesc_d)
    nc.sync.dma_start(oneone, oneone_d)
    nc.vector.memset(u_row, 0.0)
    nc.sync.dma_start(u_row[0:1, 0:_IN], xrow_d)
    nc.vector.memset(u_col, 0.0)
    nc.sync.dma_start(u_col[:, 0:2], xcol_d)

    # ---- Phase 1: K/V = A @ S (pos-blocked), kmax/kmin over frozen rows ----
    from concourse import library_config
    nc.gpsimd.load_library(library_config.attnmlp)
    ps1 = ctx.enter_context(tc.tile_pool(name="ps1", bufs=2, space="PSUM"))

    def phase1_tile(b, t):
        psk = ps1.tile([128, 256], dt, tag="psk")
        psv = ps1.tile([128, 256], dt, tag="psv")
        for ct in range(6):
            lhs = at_sb[:, ct, 128 * t:128 * (t + 1)]
            nc.tensor.matmul(psk, lhs, skv_sb[:, ct, 256 * b:256 * (b + 1)],
                             start=(ct == 0), stop=(ct == 5))
            nc.tensor.matmul(psv, lhs,
                             skv_sb[:, ct, _N + 256 * b:_N + 256 * (b + 1)],
                             start=(ct == 0), stop=(ct == 5))
        nc.scalar.copy(k_sb[b][:, t, :], psk)
        nc.vector.tensor_copy(v_sb[b][:, t, :], psv)
        if t == 0:
            nc.vector.tensor_copy(kmax[b], psk)
            nc.vector.tensor_scalar_mul(nkmin[b], psk, -1.0)
        else:
            nc.vector.tensor_max(kmax[b], kmax[b], psk)
            nc.vector.scalar_tensor_tensor(nkmin[b], psk, -1.0, nkmin[b],
                                           op0=OP.mult, op1=OP.max)

    def finish_kmm(b):
        # partition_all_reduce leaves the result on ALL partitions, so kmm
        # can be built replicated — the whole m-chain then runs on 128 lanes.
        nc.gpsimd.partition_all_reduce(kmax[b], kmax[b], channels=128,
                                       reduce_op=bass_isa.ReduceOp.max)
        nc.gpsimd.partition_all_reduce(nkmin[b], nkmin[b], channels=128,
                                       reduce_op=bass_isa.ReduceOp.max)
        nc.vector.tensor_copy(kmm[b][:, :, 0], kmax[b])
        nc.scalar.mul(kmm[b][:, :, 1], nkmin[b], -1.0)

    # static x-contribution to every node's [q|k|v]: u cols 0,1 never change,
    # so fold them out of the per-chunk matvec into one precomputed table.
    static_qkv = pool.tile([128, 3 * _N], dt, tag="stq")
    with tc.tile_pool(name="s2xp", bufs=1) as s2xp:
        s2x = s2xp.tile([128, 2, 3 * _N], dt, tag="s2x")
        for j in range(2):
            nc.sync.dma_start(s2x[:, j, :], s2_d[128 * j:128 * (j + 1), :])
        with tc.tile_pool(name="psst", bufs=1, space="PSUM") as psst:
            for seg in range(3):
                ps_st = psst.tile([128, 512], dt, tag="ps_st")
                for j in range(2):
                    nc.tensor.matmul(ps_st,
                                     u_col[:, j:j + 1].broadcast_to([128, 128]),
                                     s2x[:, j, 512 * seg:512 * (seg + 1)],
                                     start=(j == 0), stop=(j == 1))
                nc.scalar.copy(static_qkv[:, 512 * seg:512 * (seg + 1)], ps_st)

    for t in range(16):
        phase1_tile(0, t)
    finish_kmm(0)

    # ---- Phase 2: level-scheduled sequential node loop ----
    # Block-1 phase-1 tiles are interleaved between early (pos<256) chunks so
    # the PE crunches them while DVE/ACT walk the serial dependency chain.
    # Same-level chunk pairs are emitted front-first so the second chunk's
    # matvec isn't stuck behind the first chunk's late PE ops (engine FIFO).
    ps_qv = ctx.enter_context(tc.tile_pool(name="ps_qv", bufs=2, space="PSUM"))
    ps_a = ctx.enter_context(tc.tile_pool(name="ps_a", bufs=1, space="PSUM"))
    ps_b = ctx.enter_context(tc.tile_pool(name="ps_b", bufs=1, space="PSUM"))

    def chunk_front(off, B, jmax):
        # broadcast matvec over the dynamic out-columns only (x part is in
        # static_qkv): stationary u_col column replicated along free, so
        # every partition of the PSUM result holds the same [q|k|v] row.
        s2c = s2p.tile([128, 6, 3 * _CB], dt, tag="s2c", name="s2c")
        if jmax > 2:
            nc.sync.dma_start(
                s2c[:, 2:jmax, 0:3 * B],
                s2_d[256:128 * jmax, 3 * off:3 * off + 3 * B]
                .rearrange("(a p) c -> p a c", p=128))
        qkv = rowp.tile([128, 64], dt, tag="qkv_sb", name="qkv_sb")
        stq = static_qkv[:, 3 * off:3 * off + 3 * B]
        if jmax <= 2:
            nc.scalar.copy(qkv[:, 0:3 * B], stq)
        else:
            ps_qkv = ps_qv.tile([128, 64], dt, tag="qkv", name="ps_qkv")
            for j in range(2, jmax):
                nc.tensor.matmul(ps_qkv[:, 0:3 * B],
                                 u_col[:, j:j + 1].broadcast_to([128, 128]),
                                 s2c[:, j, 0:3 * B],
                                 start=(j == 2), stop=(j == jmax - 1))
            nc.vector.scalar_tensor_tensor(qkv[:, 0:3 * B], ps_qkv[:, 0:3 * B],
                                           1.0, stq, op0=OP.mult, op1=OP.add)
        return qkv

    def chunk_rest(off, B, qkv):
        blk = off // 256
        offl = off % 256
        nc.scalar.copy(k_sb[blk][0:1, 0, offl:offl + B], qkv[0:1, B:2 * B])
        nc.scalar.copy(v_sb[blk][0:1, 0, offl:offl + B], qkv[0:1, 2 * B:3 * B])

        # m = max(q*kmax, q*kmin, q*k_l), replicated on all 128 partitions
        mch = rowp.tile([128, _CB, 2], dt, tag="mch")
        qb2 = qkv[:, 0:B].unsqueeze(2).broadcast_to([128, B, 2])
        nc.vector.tensor_mul(mch[:, 0:B, :], kmm[blk][:, offl:offl + B, :], qb2)
        m12 = rowp.tile([128, _CB], dt, tag="m12")
        nc.vector.reduce_max(m12[:, 0:B], mch[:, 0:B, :], axis=AX.X)
        t3 = rowp.tile([128, _CB], dt, tag="t3")
        nc.vector.tensor_mul(t3[:, 0:B], qkv[:, 0:B], qkv[:, B:2 * B])
        mst = rowp.tile([128, _CB], dt, tag="mst")
        nc.vector.scalar_tensor_tensor(mst[:, 0:B], t3[:, 0:B], 1.0,
                                       m12[:, 0:B], op0=OP.mult, op1=OP.max)

        ksl = k_sb[blk][:, :, offl:offl + B]
        q_bc = qkv[:, 0:B].unsqueeze(1).broadcast_to([128, 16, B])
        m_bc = mst[:, 0:B].unsqueeze(1).broadcast_to([128, 16, B])
        s_scr = hv.tile([128, 16, _CB], dt, tag="s")
        nc.vector.tensor_mul(s_scr[:, :, 0:B], ksl, q_bc)
        s2_scr = hv.tile([128, 16, _CB], dt, tag="s2")
        nc.vector.tensor_sub(s2_scr[:, :, 0:B], s_scr[:, :, 0:B], m_bc)
        e_scr = hv.tile([128, 16, _CB], dt, tag="e")
        nc.scalar.activation(e_scr[:, :, 0:B], s2_scr[:, :, 0:B], AF.Exp)
        ev_scr = hv.tile([128, 16, _CB], dt, tag="s")  # reuse s slot tag
        nc.vector.tensor_mul(ev_scr[:, :, 0:B], e_scr[:, :, 0:B],
                             v_sb[blk][:, :, offl:offl + B])

        ps_dn = ps_b.tile([1, 512], dt, tag="dn")
        nc.tensor.matmul(ps_dn[0:1, 0:16 * B], onesc, e_scr[:, :, 0:B],
                         start=True, stop=True)
        nc.tensor.matmul(ps_dn[0:1, 256:256 + 16 * B], onesc, ev_scr[:, :, 0:B],
                         start=True, stop=True)
        dn = rowp.tile([1, 64], dt, tag="dn")
        pdv = (ps_dn[0:1, :].rearrange("a (u x) -> a u x", u=2)[0:1, :, 0:16 * B]
               .rearrange("a u (t b) -> a u t b", b=B))
        nc.vector.reduce_sum(
            dn[0:1, 0:2 * B], pdv.transpose([0, 1, 3, 2]), axis=AX.X)
        rc = rowp.tile([1, _CB], dt, tag="rc")
        nc.vector.reciprocal(rc[0:1, 0:B], dn[0:1, 0:B])
        rt = rowp.tile([1, _CB], dt, tag="rt")
        nc.vector.tensor_mul(rt[0:1, 0:B], dn[0:1, B:2 * B], rc[0:1, 0:B])
        nc.scalar.activation(u_row[0:1, _IN + off:_IN + off + B],
                             rt[0:1, 0:B], AF.Tanh)

        # refresh u_col blocks touched by this chunk's outputs
        j0 = (_IN + off) // 128
        j1 = (_IN + off + B - 1) // 128
        for j in range(j0, j1 + 1):
            ps_tr = ps_a.tile([128, 1], dt, tag="tr", name="ps_tr")
            nc.tensor.matmul(ps_tr, u_row[0:1, 128 * j:128 * (j + 1)], oneone,
                             start=True, stop=True)
            nc.scalar.copy(u_col[:, j:j + 1], ps_tr)

    b1_t = 0
    i = 0
    while i < len(chunks):
        off, B, jmax, lv = chunks[i]
        blk = off // 256
        if blk == 1 and b1_t <= 16:
            while b1_t < 16:
                phase1_tile(1, b1_t)
                b1_t += 1
            finish_kmm(1)
            b1_t = 17
        pair = None
        if i + 1 < len(chunks):
            off2, B2, jmax2, lv2 = chunks[i + 1]
            if lv2 == lv and off2 // 256 == blk:
                pair = (off2, B2, jmax2)
        qkv1 = chunk_front(off, B, jmax)
        qkv2 = chunk_front(*pair) if pair else None
        chunk_rest(off, B, qkv1)
        if pair:
            chunk_rest(pair[0], pair[1], qkv2)
        # slip one block-1 phase-1 tile into the PE stream per early chunk
        if blk == 0 and b1_t < 16:
            phase1_tile(1, b1_t)
            b1_t += 1
        i += 2 if pair else 1

    nc.sync.dma_start(out_d, u_row)


def make_program(x, actives, weights, in_idxs):
    import concourse.tile as tile
    from concourse import bacc

    arrays, order, pos_of, chunks = _host_prep(x, actives, weights, in_idxs)
    nc = bacc.Bacc("TRN2", target_bir_lowering=False, debug=False,
                   enable_asserts=False, num_devices=8)
    with tile.TileContext(nc) as tc:
        with ExitStack() as ctx:
            _build(nc, tc, ctx, chunks)
    nc.compile()
    return nc, arrays, pos_of


def kernel(x, actives, weights, in_idxs):
    import sys
    if "/opt/trn_rl_repo" not in sys.path:
        sys.path.insert(0, "/opt/trn_rl_repo")
    from concourse.bass_utils import run_bass_kernel_spmd

    nc, arrays, pos_of = make_program(x, actives, weights, in_idxs)
    in_maps = [dict(arrays) for _ in range(8)]
    res = run_bass_kernel_spmd(nc, in_maps, core_ids=list(range(8)))
    u = np.asarray(res.results[0]["out"]).reshape(_C)
    return u[_IN + pos_of[_N - _OUT:_N]].astype(np.float32)
